# revision 7
# baseline (speedup 1.0000x reference)
"""Trainium2 Bass kernel for nn_Bspline_19335942766607.

inputs [16, 25, 2048] f32 -> flow [16, 25, 192, 192, 2] f32.

Math: each of the 400 samples is a 32x32x2 control-point grid, bilinearly
resampled to 192x192 per channel and scaled by -192.  The query grid is
fixed, so per sample and channel this is two constant-matrix products:
    T_c = (-192 * Ay) @ P_c        Ay [192,32] interpolation matrix
    D_c = T_c @ Ax^T               Ax [192,32]

Kernel design (per core, 50 samples; pure data-parallel over 8 cores):
- fp16 two-way split arithmetic: p = p_hi + p_lo (host-split fp16) and
  tt = 3*(tt_hi + tt_lo) (on-chip split), with near-exact fp16 constants
  (-192*Ay entries are integers; 3*Ax entries are k/64), accumulating in
  fp32 PSUM.  fp16 matmuls run at 1 cycle/column vs 4 for fp32, so two
  splits cost half of fp32 while matching it to ~3e-6 relative error.
- samples processed in PAIRS via PE tile_position: sample a occupies
  column-groups 0-1 / tt rows 0:64, sample b groups 2-3 / rows 64:128.
- stage 2 produces ROW-INTERLEAVED stripes: output row r (of the pair's
  384 rows) = 3p + k lives on PSUM partition p, stripe k, via stride-3
  lhsT column selections.  The three [128, 384] stripes then form one
  fully-contiguous [128 x 4608 B] block; two pairs share one SBUF tile
  and leave in a single 1.15 MB contiguous DMA, round-robined across the
  sync / gpsimd / scalar DGE rings.
- PSUM slots: tt triple-buffered + 5 shared stripe slots (8 banks)
  so the PE never waits on the DVE lo-split to free a tt slot.
- emission is software-pipelined (stage-1 of pair j+2 and tt-split of
  pair j+1 are emitted between stage-2 and copies of pair j) so the PE
  never stalls on the ACT/DVE round trip.

Measured on 8 axon-tunneled trn2 cores: ~45-55 us/exec (output-DMA bound;
fp32 baseline of the same pipeline: ~219 us).
"""

import sys

if "/opt/trn_rl_repo" not in sys.path:
    sys.path.insert(0, "/opt/trn_rl_repo")

import numpy as np

import concourse.mybir as mybir
from concourse import bacc
from concourse.bass import ds
from concourse.bass_utils import run_bass_kernel_spmd
from concourse.tile import TileContext

F32 = mybir.dt.float32
F16 = mybir.dt.float16

B, T = 16, 25
H, W = 192, 192
G = 32
N_CORES = 8
N_SAMPLES = B * T                   # 400
S_PER_CORE = N_SAMPLES // N_CORES   # 50
FW = 2 * W                          # 384


def _interp_weights(size_out, size_in):
    q = (np.arange(size_out, dtype=np.float32) / np.float32(size_out)) * np.float32(
        size_in - 1
    )
    f = np.clip(np.floor(q), np.float32(0.0), np.float32(size_in - 2))
    idx0 = f.astype(np.int32)
    alpha = np.clip(q - f, np.float32(0.0), np.float32(1.0))
    return idx0, alpha


def _make_constants():
    """ayt16 [32,192] = fp16((-192*Ay)^T), axt3 [128,384] = fp16(3*Ax)^T
    channel-interleaved and duplicated into both partition halves."""
    y0, ay = _interp_weights(H, G)
    x0, ax = _interp_weights(W, G)
    Ay = np.zeros((H, G), dtype=np.float32)
    Ay[np.arange(H), y0] = np.float32(1.0) - ay
    Ay[np.arange(H), y0 + 1] += ay
    Ax = np.zeros((W, G), dtype=np.float32)
    Ax[np.arange(W), x0] = np.float32(1.0) - ax
    Ax[np.arange(W), x0 + 1] += ax
    ayt16 = (np.float32(-H) * Ay).T.astype(np.float16)
    ax3 = (np.float32(3.0) * Ax).T.astype(np.float16)
    axt3 = np.zeros((128, FW), dtype=np.float16)
    for c in range(2):
        axt3[c * G : (c + 1) * G, c::2] = ax3
        axt3[64 + c * G : 64 + (c + 1) * G, c::2] = ax3
    return np.ascontiguousarray(ayt16), np.ascontiguousarray(axt3)


def build(n_samples=S_PER_CORE, n_reps=1):
    """Per-core Bass program (SPMD across 8 cores)."""
    assert n_samples % 2 == 0
    npair = n_samples // 2
    nc = bacc.Bacc(None, target_bir_lowering=False, debug=False)
    # ph/pl arrive host-transposed [G, n*64] so the load is one contiguous DMA
    ph_ext = nc.declare_dram_parameter("ph", [G, n_samples * 2 * G], F16, isOutput=False)
    pl_ext = nc.declare_dram_parameter("pl", [G, n_samples * 2 * G], F16, isOutput=False)
    ayt_ext = nc.declare_dram_parameter("ayt16", [G, H], F16, isOutput=False)
    axt_ext = nc.declare_dram_parameter("axt3", [128, FW], F16, isOutput=False)
    out_ext = nc.declare_dram_parameter(
        "out", [n_samples, H, FW], F32, isOutput=True
    )
    dma_batch = 2

    with TileContext(nc) as tc:
        with (
            tc.tile_pool(name="const", bufs=1) as cpool,
            tc.tile_pool(name="work", bufs=4) as wpool,
            tc.tile_pool(name="psum", bufs=1, space="PSUM") as pspool,
        ):
            ayt_sb = cpool.tile([G, H], F16)
            nc.sync.dma_start(out=ayt_sb[:], in_=ayt_ext[:])
            axt_sb = cpool.tile([128, FW], F16)
            nc.sync.dma_start(out=axt_sb[:], in_=axt_ext[:])
            ph_sb = cpool.tile([G, n_samples * 2 * G], F16)
            nc.sync.dma_start(out=ph_sb[:], in_=ph_ext[:])
            pl_sb = cpool.tile([G, n_samples * 2 * G], F16)
            nc.sync.dma_start(out=pl_sb[:], in_=pl_ext[:])

            dma_cycle = [nc.sync, nc.gpsimd, nc.scalar]

            for _rep in range(n_reps):

                def s1(j):
                    # both samples' stage-1 into one [128, 192] psum tile,
                    # (hi, lo) fp16 matmuls accumulating in fp32
                    tt_ps = pspool.tile([128, H], F32, tag="tt", bufs=3, name="tt_ps")
                    for base, tp in ((0, (0, 0)), (64, (0, 64))):
                        i = 2 * j + (base // 64)
                        sl = ds(i * 2 * G, 2 * G)
                        nc.tensor.matmul(
                            tt_ps[base : base + 64], ph_sb[:, sl], ayt_sb[:],
                            start=True, stop=False, tile_position=tp,
                        )
                        nc.tensor.matmul(
                            tt_ps[base : base + 64], pl_sb[:, sl], ayt_sb[:],
                            start=False, stop=True, tile_position=tp,
                        )
                    return tt_ps

                def ctt(tt_ps):
                    # hi = fp16(tt/3) on ACT; lo = fp16(tt/3 - hi) on DVE
                    hi = wpool.tile([128, H], F16, tag="tth")
                    nc.scalar.activation(
                        hi[:], tt_ps[:],
                        mybir.ActivationFunctionType.Copy, scale=1.0 / 3.0,
                    )
                    lo = wpool.tile([128, H], F16, tag="ttl")
                    nc.vector.scalar_tensor_tensor(
                        lo[:], tt_ps[:], 1.0 / 3.0, hi[:],
                        mybir.AluOpType.mult, mybir.AluOpType.subtract,
                    )
                    return hi, lo

                def s2(tt):
                    # stripe k holds pair-output rows r = 3p + k; rows < 192
                    # are sample a (tt parts 0:64, lhsT cols k::3), rows >=
                    # 192 sample b (parts 64:128, cols k::3).
                    hi, lo = tt
                    ps = []
                    for k in range(3):
                        pk = pspool.tile([128, FW], F32, tag="pk", bufs=5, name="pk")
                        for t, stop in ((hi, False), (lo, True)):
                            nc.tensor.matmul(
                                pk[0:64], t[0:64, k : H : 3], axt_sb[0:64],
                                start=not stop, stop=stop, tile_position=(0, 0),
                            )
                        for t, stop in ((hi, False), (lo, True)):
                            nc.tensor.matmul(
                                pk[64:128], t[64:128, k : H : 3], axt_sb[64:128],
                                start=not stop, stop=stop, tile_position=(64, 64),
                            )
                        ps.append(pk)
                    return ps

                o_sb_cur = [None]

                def emit_out(j, psums):
                    bi = j % dma_batch
                    if bi == 0:
                        o_sb_cur[0] = wpool.tile(
                            [128, dma_batch * 3 * FW], F32, tag="o_sb", name="o_sb"
                        )
                    o_sb = o_sb_cur[0]
                    off = bi * 3 * FW
                    for k in range(3):
                        dst = o_sb[:, off + k * FW : off + (k + 1) * FW]
                        if k == 1:
                            nc.scalar.copy(out=dst, in_=psums[k][:])
                        else:
                            nc.vector.tensor_copy(out=dst, in_=psums[k][:])
                    if bi == dma_batch - 1 or j == npair - 1:
                        nb = bi + 1
                        s = 2 * (j - bi)
                        eng = dma_cycle[(j // dma_batch) % len(dma_cycle)]
                        # DRAM row (384*jj + 3p + k) <- o_sb[p, jj*1152+k*384+wc]
                        dst = (
                            out_ext[s : s + 2 * nb]
                            .rearrange("s h f -> (s h) f")
                            .rearrange("(jj p k) f -> p jj k f", p=128, k=3)
                            .rearrange("p jj k f -> p jj (k f)")
                        )
                        src = o_sb[:, 0 : nb * 3 * FW].rearrange(
                            "p (jj kf) -> p jj kf", jj=nb
                        )
                        eng.dma_start(out=dst, in_=src)

                tt_ps_q = {0: s1(0)}
                tt_sb_q = {0: ctt(tt_ps_q.pop(0))}
                if npair > 1:
                    tt_ps_q[1] = s1(1)
                for j in range(npair):
                    psums = s2(tt_sb_q.pop(j))
                    if j + 1 < npair:
                        tt_sb_q[j + 1] = ctt(tt_ps_q.pop(j + 1))
                    if j + 2 < npair:
                        tt_ps_q[j + 2] = s1(j + 2)
                    emit_out(j, psums)
    nc.finalize()
    return nc


_CACHE = {}


def _get_nc(n_reps=1):
    if n_reps not in _CACHE:
        _CACHE[n_reps] = build(n_reps=n_reps)
    return _CACHE[n_reps]


def prep_inputs(p_full):
    """p_full [400, 32, 64] f32 (raw [g, (g',c)]) -> per-core in_maps."""
    ayt16, axt3 = _make_constants()
    # deinterleave channels: column m = c*32 + g'
    p_d = (
        p_full.reshape(N_SAMPLES, G, G, 2)
        .transpose(0, 1, 3, 2)
        .reshape(N_SAMPLES, G, 2 * G)
    )
    hi = p_d.astype(np.float16)
    lo = (p_d - hi.astype(np.float32)).astype(np.float16)
    # host transpose to [core, G, 50*64] (partition-major, contiguous load)
    def tr(x):
        return np.ascontiguousarray(
            x.reshape(N_CORES, S_PER_CORE, G, 2 * G)
            .transpose(0, 2, 1, 3)
            .reshape(N_CORES, G, S_PER_CORE * 2 * G)
        )

    hi_t, lo_t = tr(hi), tr(lo)
    return [
        {"ph": hi_t[c], "pl": lo_t[c], "ayt16": ayt16, "axt3": axt3}
        for c in range(N_CORES)
    ]


def run_on_hw(p_full, n_reps=1):
    """p_full [400, 32, 64] f32 -> out [400, 192, 384] f32."""
    in_maps = prep_inputs(p_full)
    nc = _get_nc(n_reps)
    res = run_bass_kernel_spmd(nc, in_maps, list(range(N_CORES))).results
    out = np.stack([res[c]["out"] for c in range(N_CORES)])
    return out.reshape(N_SAMPLES, H, FW)


def kernel(inputs):
    inputs = np.ascontiguousarray(np.asarray(inputs), dtype=np.float32)
    assert inputs.shape == (B, T, 2 * G * G), inputs.shape
    out = run_on_hw(inputs.reshape(N_SAMPLES, G, 2 * G))
    return out.reshape(B, T, H, W, 2)
